# revision 34
# baseline (speedup 1.0000x reference)
"""Multi-head self-attention (B=4, T=2048, C=1024, 16 heads x hd=64) on 8
Trainium2 NeuronCores.

Sharding: tensor-parallel over heads — each core owns 2 heads (128 of the
1024 channels): its slices of Wq/Wk/Wv rows and Wo columns. Every core reads
the full x (transposed + bf16-cast on host), computes Q^T/K^T (channel-major)
and V (token-major via PE transpose) for its heads, runs attention entirely
from SBUF, then produces a rank-128 partial of the output projection. The 8
partials are summed on host (+ bo).

Per-core dataflow (all matmuls bf16 in / fp32 PSUM accumulate):
  phase 1: per 512-row block: Q^T = Wq_c @ x^T (+bq), K^T = Wk_c @ x^T (bk
           dropped — its score term q.bk is constant along the softmax
           axis), V = x @ Wv_c^T token-major with a ones column appended
           per head (denominator trick). All SBUF-resident.
  phase 2: per (batch, 512-query block): S^T [128k, 1024(2 k-tiles)] per head
           via K^T-stationary matmuls (contraction d=64), one exp per k-tile
           pair on ScalarE (scale=1/8 folded in) -> P^T bf16, then
           O^T[65,512] += [V|1]^T P^T accumulated over k with K=128 matmuls.
           Softmax denominator lands in row 64; its reciprocal is taken
           single-lane on VectorE, broadcast down 64 partitions with one
           zero-stride DMA; the normalize mul runs on GpSimd (bv rides
           through Wo on the host side as a constant row vector).
  phase 3: partial_out[128 rows, 512] = O^T-slice-stationary matmuls against
           Wo_c^T; fp16 partials DMA'd out.

Schedule: 3 projection blocks upfront; remaining 13 blocks are fillers for
global q-blocks 0..12; each q-block also drains the output projection of
the q-block TWO back (so its ot rows are long past the normalize chain),
leaving only the last two q-blocks' projections for the tail.
"""
import json

import numpy as np
import ml_dtypes

import concourse.bass as bass
import concourse.mybir as mybir
import concourse.tile as tile
from concourse.bass_utils import run_bass_kernel_spmd

bf16 = ml_dtypes.bfloat16
dt = mybir.dt

EMB = 1024
HEADS = 16
HD = 64
B = 4
T = 2048
R = B * T            # 8192 rows
NCORES = 8
F = EMB // NCORES    # 128 channels (2 heads) per core
NH = F // HD         # 2 heads per core
NKC = EMB // 128     # 8 contraction chunks for projections
NQB = T // 512       # 4 query blocks per batch
NJP = T // 256       # 8 k-tile PAIRS per batch
G = R // 128         # 64 global row/key tiles
VW = HD + 1          # 65: V head slice + ones column
NBLK = R // 512      # 16 projection row-blocks
PROLOGUE_BLOCKS = 3


# ---------------------------------------------------------------------------
# walrus in this container accepts only ONE sync-wait per instruction; split
# extra waits onto same-engine NoOps at BIR-serialization time.
_orig_to_json_bytes = bass.Bass.to_json_bytes


def _split_waits(data: bytes) -> bytes:
    d = json.loads(data)
    changed = False
    for f in d.get("functions", []):
        for blk in f.get("blocks", []):
            out = []
            for inst in blk.get("instructions", []):
                si = inst.get("sync_info")
                waits = (si or {}).get("on_wait") or []
                if len(waits) > 1:
                    changed = True
                    for i, w in enumerate(waits[:-1]):
                        out.append({
                            "debug": inst.get("debug", 0),
                            "engine": inst["engine"],
                            "ins": [], "outs": [],
                            "name": f"{inst['name']}_w{i}",
                            "opcode": "NoOp",
                            "sync_info": {"on_update": [], "on_wait": [w]},
                            "text_hint": "wait_split",
                        })
                    si["on_wait"] = waits[-1:]
                out.append(inst)
            blk["instructions"] = out
    return json.dumps(d).encode() if changed else data


def _to_json_bytes(self, *a, **k):
    return _split_waits(_orig_to_json_bytes(self, *a, **k))


bass.Bass.to_json_bytes = _to_json_bytes
# ---------------------------------------------------------------------------


def build_bass() -> bass.Bass:
    nc = bass.Bass()
    xt_ext = nc.declare_dram_parameter("xt", [EMB, R], dt.bfloat16, isOutput=False)
    wq_ext = nc.declare_dram_parameter("wq", [EMB, F], dt.bfloat16, isOutput=False)
    wk_ext = nc.declare_dram_parameter("wk", [EMB, F], dt.bfloat16, isOutput=False)
    wv_ext = nc.declare_dram_parameter("wv", [EMB, F], dt.bfloat16, isOutput=False)
    wo_ext = nc.declare_dram_parameter("wo", [F, EMB], dt.bfloat16, isOutput=False)
    bq_ext = nc.declare_dram_parameter("bq", [F, 1], dt.float32, isOutput=False)
    out_ext = nc.declare_dram_parameter("out", [R, EMB], dt.float16, isOutput=True)

    Exp = mybir.ActivationFunctionType.Exp

    with tile.TileContext(nc) as tc:
        with (
            tc.tile_pool(name="const", bufs=1) as cp,
            tc.tile_pool(name="res", bufs=1) as res,
            tc.tile_pool(name="xt", bufs=2) as xp,
            tc.tile_pool(name="pt", bufs=8) as ptp,
            tc.tile_pool(name="norm", bufs=2) as npl,
            tc.tile_pool(name="osb", bufs=3) as op,
            tc.tile_pool(name="ps", bufs=1, space="PSUM") as ps,
        ):
            # --- constants ---
            wq_sb = cp.tile([128, EMB], dt.bfloat16, tag="wq")
            wk_sb = cp.tile([128, EMB], dt.bfloat16, tag="wk")
            wv_sb = cp.tile([128, EMB], dt.bfloat16, tag="wv")
            wo_sb = cp.tile([128, EMB], dt.bfloat16, tag="wo")
            bq_sb = cp.tile([F, 1], dt.float32, tag="bq")
            def _wload(ext, tile_sb):
                nc.sync.dma_start(
                    tile_sb[:].rearrange("p (kc f) -> p kc f", f=F),
                    ext[:].rearrange("(kc p) f -> p kc f", p=128),
                )
            _wload(wq_ext, wq_sb)
            nc.sync.dma_start(bq_sb[:], bq_ext[:])
            _wload(wk_ext, wk_sb)
            _wload(wv_ext, wv_sb)
            nc.sync.dma_start(wo_sb[:], wo_ext[:])

            # --- residents ---
            qt_sb = res.tile([F, R], dt.bfloat16, tag="qt")
            kt_sb = res.tile([F, R], dt.bfloat16, tag="kt")
            ot_sb = res.tile([F, R], dt.bfloat16, tag="ot")
            va_sb = res.tile([128, G * NH * VW], dt.bfloat16, tag="va")
            nc.vector.memset(
                va_sb[:].rearrange("p (g d) -> p g d", d=VW)[:, :, HD:VW], 1.0
            )

            # ---- phase-1 emitters (one 512-row block = 5 filler units) ----
            def p1_load(rb, eng=None):
                xts = []
                for kc in range(NKC):
                    xt = xp.tile([128, 512], dt.bfloat16, tag=f"xt{kc}",
                                 name=f"xt{kc}_{rb}")
                    (eng or nc.sync).dma_start(
                        xt[:],
                        xt_ext[kc * 128:(kc + 1) * 128, rb * 512:rb * 512 + 512],
                    )
                    xts.append(xt)
                return xts

            def p1_qk(rb, xts, w_sb, dst_sb, bias):
                r0 = rb * 512
                acc = ps.tile([128, 512], dt.float32, tag="pp", bufs=2,
                              name=f"prj_{rb}_{id(w_sb)}")
                for kc in range(NKC):
                    nc.tensor.matmul(
                        acc[:], w_sb[:, kc * F:(kc + 1) * F], xts[kc][:],
                        start=(kc == 0), stop=(kc == NKC - 1),
                    )
                if bias is not None:
                    nc.vector.tensor_scalar_add(dst_sb[:, r0:r0 + 512], acc[:], bias[:])
                else:
                    nc.vector.tensor_copy(dst_sb[:, r0:r0 + 512], acc[:])

            def p1_v(rb, xts, sub):
                g = rb * 4 + sub
                acc = ps.tile([128, F], dt.float32, tag="pp", bufs=2,
                              name=f"vprj_{g}")
                for kc in range(NKC):
                    nc.tensor.matmul(
                        acc[:],
                        xts[kc][:, sub * 128:(sub + 1) * 128],
                        wv_sb[:, kc * F:(kc + 1) * F],
                        start=(kc == 0), stop=(kc == NKC - 1),
                    )
                dst = va_sb[:, g * NH * VW:(g + 1) * NH * VW].rearrange(
                    "p (h d) -> p h d", d=VW
                )[:, :, 0:HD]
                nc.vector.tensor_copy(
                    dst, acc[:].rearrange("p (h d) -> p h d", d=HD)
                )

            def p1_block_fillers(rb):
                """7 filler closures for one 512-row projection block."""
                state = {}

                def load():
                    state["xts"] = p1_load(rb)

                fillers = [load,
                           lambda: p1_qk(rb, state["xts"], wq_sb, qt_sb, bq_sb),
                           lambda: p1_qk(rb, state["xts"], wk_sb, kt_sb, None)]
                for sub in range(4):
                    fillers.append(lambda s=sub: p1_v(rb, state["xts"], s))
                return fillers

            # ---- phase-3 emitter (one 128-row tile) ----
            def p3_tile(g, act_drain=True, out_eng=None):
                """Drain one PSUM half on the ACT engine (Copy is in every
                table set — no reload) so the two halves' drains run in
                parallel and the DVE queue stays short. Skipped in the late
                q-blocks, where the denser p3 schedule would overload ACT
                next to its exp stream."""
                o_sb = op.tile([128, EMB], dt.float16, tag="osb", name=f"o_{g}")
                for ch in range(2):
                    o_ps = ps.tile([128, 512], dt.float32, tag="pp", bufs=2,
                                   name=f"ops_{g}_{ch}")
                    nc.tensor.matmul(
                        o_ps[:],
                        ot_sb[:, g * 128:(g + 1) * 128],
                        wo_sb[:, ch * 512:(ch + 1) * 512],
                        start=True, stop=True,
                    )
                    dst = o_sb[:, ch * 512:(ch + 1) * 512]
                    if ch == 1 and act_drain:
                        nc.scalar.activation(
                            dst, o_ps[:], mybir.ActivationFunctionType.Copy
                        )
                    else:
                        nc.vector.tensor_copy(dst, o_ps[:])
                (out_eng or nc.sync).dma_start(
                    out_ext[g * 128:(g + 1) * 128, :], o_sb[:])

            # ---- phase-2 q-block with interleaved fillers ----
            def p2_qblock(b, qb, fillers, late=False):
                q0 = b * T + qb * 512
                fi = iter(fillers)

                def fill(n=1):
                    for _ in range(n):
                        f = next(fi, None)
                        if f is not None:
                            f()

                pvs = {h: ps.tile([VW, 512], dt.float32, tag="pv", bufs=2,
                                  name=f"pv_{b}_{qb}_{h}")
                       for h in range(NH)}
                pts = {}

                def emit_st(jp):
                    k0 = b * T + jp * 256
                    for h in range(NH):
                        st = ps.tile([128, 1024], dt.float32, tag="st", bufs=2,
                                     name=f"st_{b}_{qb}_{jp}_{h}")
                        for half in range(2):
                            nc.tensor.matmul(
                                st[:, half * 512:(half + 1) * 512],
                                kt_sb[h * HD:(h + 1) * HD,
                                      k0 + half * 128:k0 + (half + 1) * 128],
                                qt_sb[h * HD:(h + 1) * HD, q0:q0 + 512],
                                start=True, stop=True,
                            )
                        pt = ptp.tile([128, 1024], dt.bfloat16, tag="pt",
                                      name=f"pt_{b}_{qb}_{jp}_{h}")
                        nc.scalar.activation(pt[:], st[:], Exp, scale=0.125)
                        pts[(jp, h)] = pt

                def emit_pv(jp):
                    g0 = b * NJP * 2 + jp * 2
                    for h in range(NH):
                        pt = pts.pop((jp, h))
                        for half in range(2):
                            g = g0 + half
                            va = va_sb[:, g * NH * VW + h * VW:
                                       g * NH * VW + (h + 1) * VW]
                            nc.tensor.matmul(
                                pvs[h][:], va[:],
                                pt[:, half * 512:(half + 1) * 512],
                                start=(jp == 0 and half == 0),
                                stop=(jp == NJP - 1 and half == 1),
                            )

                for jp in range(NJP):
                    emit_st(jp)
                    fill(1)
                    if jp > 1:
                        emit_pv(jp - 2)
                emit_pv(NJP - 2)
                fill(1)
                emit_pv(NJP - 1)
                # copy both heads' PV to SBUF first (frees PSUM), launch the
                # denominator-reshape DMAs immediately after each copy, and
                # only then run the reciprocals: by the time the in-order DVE
                # queue reaches recip h0, its d4 DMA has landed — the DVE
                # must never sit waiting on a DMA, since that head-of-line
                # blocking stalls the PSUM-recycling casts the PE filler
                # matmuls depend on
                ms = {}
                d4s = {}
                for h in range(NH):
                    m = npl.tile([VW, 512], dt.float32, tag="m", name=f"m_{b}_{qb}_{h}")
                    nc.vector.tensor_copy(m[:], pvs[h][:])
                    d4 = npl.tile([128, 4], dt.float32, tag="d4",
                                  name=f"d4_{b}_{qb}_{h}")
                    nc.sync.dma_start(
                        d4[:],
                        m[HD:VW, :].rearrange("p (a c) -> p a c", c=4),
                    )
                    ms[h] = m
                    d4s[h] = d4
                fill(2)
                for h in range(NH):
                    m = ms[h]
                    r4 = npl.tile([128, 4], dt.float32, tag="r4", name=f"r4_{b}_{qb}_{h}")
                    nc.vector.reciprocal(r4[:], d4s[h][:])
                    rc = npl.tile([1, 512], dt.float32, tag="rc", name=f"rc_{b}_{qb}_{h}")
                    nc.sync.dma_start(
                        rc[:].rearrange("p (a c) -> p a c", c=4), r4[:]
                    )
                    rbt = npl.tile([HD, 512], dt.float32, tag="rb", name=f"rb_{b}_{qb}_{h}")
                    nc.sync.dma_start(
                        rbt[:],
                        rc[0:1, :].rearrange("p (o q) -> p o q", o=1)
                        .broadcast_to((1, HD, 512)),
                    )
                    # bv is folded into the host-side gather (bv @ Wo^T is
                    # a constant row vector), so the normalize is a single
                    # mul. GpSimd mid-kernel (keeps the DVE queue unblocked);
                    # DVE in the last q-blocks, where the GpSimd wake-up
                    # latency would sit on the tail critical path.
                    osl = ot_sb[h * HD:(h + 1) * HD, q0:q0 + 512]
                    eng = nc.vector if late else nc.gpsimd
                    eng.tensor_mul(osl, m[0:HD, :], rbt[:])
                fill(100)   # drain any leftover fillers

            # ---------------- emission schedule ----------------
            # prologue: first 3 projection blocks upfront
            for rb in range(PROLOGUE_BLOCKS):
                for f in p1_block_fillers(rb):
                    f()
            # p3 tiles per q-block: lag >= 2 q-blocks behind (so ot rows are
            # long past the normalize chain); back-load the distribution so
            # the last batch — which has no projection fillers left — still
            # keeps the PE busy.
            P3_COUNT = [0, 0, 4, 4, 4, 4, 4, 4, 4, 4, 4, 4, 5, 5, 5, 5]
            nxt_g = 0
            for b in range(B):
                for qb in range(NQB):
                    i = b * NQB + qb          # global q-block index 0..15
                    fillers = []
                    blk = PROLOGUE_BLOCKS + i  # p1 block for this q-block
                    if blk < NBLK:
                        fillers.extend(p1_block_fillers(blk))
                    hi = min(nxt_g + P3_COUNT[i], (i - 1) * 4 if i > 1 else 0)
                    for g in range(nxt_g, hi):
                        fillers.append(lambda g=g: p3_tile(g, act_drain=False))
                    nxt_g = max(nxt_g, hi)
                    p2_qblock(b, qb, fillers, late=(i >= NQB * B - 2))
            # tail: whatever output projection remains (last two q-blocks)
            for g in range(nxt_g, G):
                p3_tile(g, act_drain=True)
    return nc


_NC_CACHE = None


def _get_nc():
    global _NC_CACHE
    if _NC_CACHE is None:
        _NC_CACHE = build_bass()
    return _NC_CACHE


def make_in_maps(x, Wq, bq, Wk, bk, Wv, bv, Wo, bo):
    xt = np.ascontiguousarray(
        np.asarray(x, dtype=np.float32).reshape(R, EMB).astype(bf16).T
    )
    in_maps = []
    for c in range(NCORES):
        rows = slice(F * c, F * (c + 1))
        in_maps.append({
            "xt": xt,
            "wq": np.ascontiguousarray(np.asarray(Wq)[rows, :].T.astype(bf16)),
            "wk": np.ascontiguousarray(np.asarray(Wk)[rows, :].T.astype(bf16)),
            "wv": np.ascontiguousarray(np.asarray(Wv)[rows, :].T.astype(bf16)),
            "wo": np.ascontiguousarray(np.asarray(Wo)[:, rows].T.astype(bf16)),
            "bq": np.asarray(bq)[rows].reshape(F, 1).astype(np.float32),
        })
    return in_maps


def gather(results, bv, Wo, bo):
    acc = np.zeros((R, EMB), np.float32)
    for r in results:
        acc += r["out"].astype(np.float32)
    # bv rides through the output projection as a constant row vector
    acc += np.asarray(bo, dtype=np.float32) + (
        np.asarray(bv, dtype=np.float64) @ np.asarray(Wo, dtype=np.float64).T
    ).astype(np.float32)
    return acc.reshape(B, T, EMB)


def kernel(x, Wq, bq, Wk, bk, Wv, bv, Wo, bo, _trace=False):
    nc = _get_nc()
    in_maps = make_in_maps(x, Wq, bq, Wk, bk, Wv, bv, Wo, bo)
    res = run_bass_kernel_spmd(nc, in_maps, list(range(NCORES)), trace=_trace)
    out = gather(res.results, bv, Wo, bo)
    if _trace:
        kernel.last_result = res
    return out


# revision 38
# speedup vs baseline: 1.0385x; 1.0385x over previous
"""Multi-head self-attention (B=4, T=2048, C=1024, 16 heads x hd=64) on 8
Trainium2 NeuronCores.

Sharding: tensor-parallel over heads — each core owns 2 heads (128 of the
1024 channels): its slices of Wq/Wk/Wv rows and Wo columns. Every core reads
the full x (transposed + bf16-cast on host), computes Q^T/K^T (channel-major)
and V (token-major via PE transpose) for its heads, runs attention entirely
from SBUF, then produces a rank-128 partial of the output projection. The 8
partials are summed on host (+ bo).

Per-core dataflow (all matmuls bf16 in / fp32 PSUM accumulate):
  phase 1: per 512-row block: Q^T = Wq_c @ x^T (+bq), K^T = Wk_c @ x^T (bk
           dropped — its score term q.bk is constant along the softmax
           axis), V = x @ Wv_c^T token-major with a ones column appended
           per head (denominator trick). All SBUF-resident.
  phase 2: per (batch, 512-query block): S^T [128k, 1024(2 k-tiles)] per head
           via K^T-stationary matmuls (contraction d=64), one exp per k-tile
           pair on ScalarE (scale=1/8 folded in) -> P^T bf16, then
           O^T[65,512] += [V|1]^T P^T accumulated over k with K=128 matmuls.
           Softmax denominator lands in row 64; its reciprocal is taken
           single-lane on VectorE, broadcast down 64 partitions with one
           zero-stride DMA; the normalize mul runs on GpSimd (bv rides
           through Wo on the host side as a constant row vector).
  phase 3: partial_out[128 rows, 512] = O^T-slice-stationary matmuls against
           Wo_c^T; fp16 partials DMA'd out.

Schedule: 3 projection blocks upfront; remaining 13 blocks are fillers for
global q-blocks 0..12; each q-block also drains the output projection of
the q-block TWO back (so its ot rows are long past the normalize chain),
leaving only the last two q-blocks' projections for the tail.
"""
import json

import numpy as np
import ml_dtypes

import concourse.bass as bass
import concourse.mybir as mybir
import concourse.tile as tile
from concourse.bass_utils import run_bass_kernel_spmd

bf16 = ml_dtypes.bfloat16
dt = mybir.dt

EMB = 1024
HEADS = 16
HD = 64
B = 4
T = 2048
R = B * T            # 8192 rows
NCORES = 8
F = EMB // NCORES    # 128 channels (2 heads) per core
NH = F // HD         # 2 heads per core
NKC = EMB // 128     # 8 contraction chunks for projections
NQB = T // 512       # 4 query blocks per batch
NJP = T // 256       # 8 k-tile PAIRS per batch
G = R // 128         # 64 global row/key tiles
VW = HD + 1          # 65: V head slice + ones column
NBLK = R // 512      # 16 projection row-blocks
PROLOGUE_BLOCKS = 3


# ---------------------------------------------------------------------------
# walrus in this container accepts only ONE sync-wait per instruction; split
# extra waits onto same-engine NoOps at BIR-serialization time.
_orig_to_json_bytes = bass.Bass.to_json_bytes


def _split_waits(data: bytes) -> bytes:
    d = json.loads(data)
    changed = False
    for f in d.get("functions", []):
        for blk in f.get("blocks", []):
            out = []
            for inst in blk.get("instructions", []):
                si = inst.get("sync_info")
                waits = (si or {}).get("on_wait") or []
                if len(waits) > 1:
                    changed = True
                    for i, w in enumerate(waits[:-1]):
                        out.append({
                            "debug": inst.get("debug", 0),
                            "engine": inst["engine"],
                            "ins": [], "outs": [],
                            "name": f"{inst['name']}_w{i}",
                            "opcode": "NoOp",
                            "sync_info": {"on_update": [], "on_wait": [w]},
                            "text_hint": "wait_split",
                        })
                    si["on_wait"] = waits[-1:]
                out.append(inst)
            blk["instructions"] = out
    return json.dumps(d).encode() if changed else data


def _to_json_bytes(self, *a, **k):
    return _split_waits(_orig_to_json_bytes(self, *a, **k))


bass.Bass.to_json_bytes = _to_json_bytes
# ---------------------------------------------------------------------------


def build_bass() -> bass.Bass:
    nc = bass.Bass()
    xt_ext = nc.declare_dram_parameter("xt", [EMB, R], dt.bfloat16, isOutput=False)
    wq_ext = nc.declare_dram_parameter("wq", [EMB, F], dt.bfloat16, isOutput=False)
    wk_ext = nc.declare_dram_parameter("wk", [EMB, F], dt.bfloat16, isOutput=False)
    wv_ext = nc.declare_dram_parameter("wv", [EMB, F], dt.bfloat16, isOutput=False)
    wo_ext = nc.declare_dram_parameter("wo", [F, EMB], dt.bfloat16, isOutput=False)
    bq_ext = nc.declare_dram_parameter("bq", [F, 1], dt.float32, isOutput=False)
    out_ext = nc.declare_dram_parameter("out", [R, EMB], dt.float16, isOutput=True)

    Exp = mybir.ActivationFunctionType.Exp

    with tile.TileContext(nc) as tc:
        with (
            tc.tile_pool(name="const", bufs=1) as cp,
            tc.tile_pool(name="res", bufs=1) as res,
            tc.tile_pool(name="xt", bufs=2) as xp,
            tc.tile_pool(name="pt", bufs=6) as ptp,
            tc.tile_pool(name="norm", bufs=2) as npl,
            tc.tile_pool(name="osb", bufs=3) as op,
            tc.tile_pool(name="ps", bufs=1, space="PSUM") as ps,
        ):
            # --- constants ---
            wq_sb = cp.tile([128, EMB], dt.bfloat16, tag="wq")
            wk_sb = cp.tile([128, EMB], dt.bfloat16, tag="wk")
            wv_sb = cp.tile([128, EMB], dt.bfloat16, tag="wv")
            wo_sb = cp.tile([128, EMB], dt.bfloat16, tag="wo")
            bq_sb = cp.tile([F, 1], dt.float32, tag="bq")
            def _wload(ext, tile_sb):
                nc.sync.dma_start(
                    tile_sb[:].rearrange("p (kc f) -> p kc f", f=F),
                    ext[:].rearrange("(kc p) f -> p kc f", p=128),
                )
            _wload(wq_ext, wq_sb)
            nc.sync.dma_start(bq_sb[:], bq_ext[:])
            _wload(wk_ext, wk_sb)
            _wload(wv_ext, wv_sb)
            nc.sync.dma_start(wo_sb[:], wo_ext[:])

            # --- residents ---
            qt_sb = res.tile([F, R], dt.bfloat16, tag="qt")
            kt_sb = res.tile([F, R], dt.bfloat16, tag="kt")
            ot_sb = res.tile([F, R], dt.bfloat16, tag="ot")
            va_sb = res.tile([128, G * NH * VW], dt.bfloat16, tag="va")
            nc.vector.memset(
                va_sb[:].rearrange("p (g d) -> p g d", d=VW)[:, :, HD:VW], 1.0
            )

            # ---- phase-1 emitters (one 512-row block = 5 filler units) ----
            def p1_load(rb, eng=None):
                xts = []
                for kc in range(NKC):
                    xt = xp.tile([128, 512], dt.bfloat16, tag=f"xt{kc}",
                                 name=f"xt{kc}_{rb}")
                    (eng or nc.sync).dma_start(
                        xt[:],
                        xt_ext[kc * 128:(kc + 1) * 128, rb * 512:rb * 512 + 512],
                    )
                    xts.append(xt)
                return xts

            def p1_qk(rb, xts, w_sb, dst_sb, bias):
                r0 = rb * 512
                acc = ps.tile([128, 512], dt.float32, tag="pp", bufs=2,
                              name=f"prj_{rb}_{id(w_sb)}")
                for kc in range(NKC):
                    nc.tensor.matmul(
                        acc[:], w_sb[:, kc * F:(kc + 1) * F], xts[kc][:],
                        start=(kc == 0), stop=(kc == NKC - 1),
                    )
                if bias is not None:
                    nc.vector.tensor_scalar_add(dst_sb[:, r0:r0 + 512], acc[:], bias[:])
                else:
                    nc.vector.tensor_copy(dst_sb[:, r0:r0 + 512], acc[:])

            def p1_v(rb, xts, sub):
                g = rb * 4 + sub
                acc = ps.tile([128, F], dt.float32, tag="pp", bufs=2,
                              name=f"vprj_{g}")
                for kc in range(NKC):
                    nc.tensor.matmul(
                        acc[:],
                        xts[kc][:, sub * 128:(sub + 1) * 128],
                        wv_sb[:, kc * F:(kc + 1) * F],
                        start=(kc == 0), stop=(kc == NKC - 1),
                    )
                dst = va_sb[:, g * NH * VW:(g + 1) * NH * VW].rearrange(
                    "p (h d) -> p h d", d=VW
                )[:, :, 0:HD]
                nc.vector.tensor_copy(
                    dst, acc[:].rearrange("p (h d) -> p h d", d=HD)
                )

            def p1_block_fillers(rb):
                """7 filler closures for one 512-row projection block."""
                state = {}

                def load():
                    state["xts"] = p1_load(rb)

                fillers = [load,
                           lambda: p1_qk(rb, state["xts"], wq_sb, qt_sb, bq_sb),
                           lambda: p1_qk(rb, state["xts"], wk_sb, kt_sb, None)]
                for sub in range(4):
                    fillers.append(lambda s=sub: p1_v(rb, state["xts"], s))
                return fillers

            # ---- phase-3 emitter (one 128-row tile) ----
            def p3_tile(g, act_drain=True, out_eng=None):
                """Drain one PSUM half on the ACT engine (Copy is in every
                table set — no reload) so the two halves' drains run in
                parallel and the DVE queue stays short. Skipped in the late
                q-blocks, where the denser p3 schedule would overload ACT
                next to its exp stream."""
                o_sb = op.tile([128, EMB], dt.float16, tag="osb", name=f"o_{g}")
                for ch in range(2):
                    o_ps = ps.tile([128, 512], dt.float32, tag="pp", bufs=2,
                                   name=f"ops_{g}_{ch}")
                    nc.tensor.matmul(
                        o_ps[:],
                        ot_sb[:, g * 128:(g + 1) * 128],
                        wo_sb[:, ch * 512:(ch + 1) * 512],
                        start=True, stop=True,
                    )
                    dst = o_sb[:, ch * 512:(ch + 1) * 512]
                    if ch == 1 and act_drain:
                        nc.scalar.activation(
                            dst, o_ps[:], mybir.ActivationFunctionType.Copy
                        )
                    else:
                        nc.vector.tensor_copy(dst, o_ps[:])
                (out_eng or nc.sync).dma_start(
                    out_ext[g * 128:(g + 1) * 128, :], o_sb[:])

            # ---- phase-2 q-block with interleaved fillers ----
            def p2_qblock(b, qb, fillers, late=False, last=False):
                q0 = b * T + qb * 512
                fi = iter(fillers)

                def fill(n=1):
                    for _ in range(n):
                        f = next(fi, None)
                        if f is not None:
                            f()

                pvs = {h: ps.tile([VW, 512], dt.float32, tag="pv", bufs=2,
                                  name=f"pv_{b}_{qb}_{h}")
                       for h in range(NH)}
                pts = {}

                def emit_st(jp):
                    k0 = b * T + jp * 256
                    for h in range(NH):
                        st = ps.tile([128, 1024], dt.float32, tag="st", bufs=2,
                                     name=f"st_{b}_{qb}_{jp}_{h}")
                        for half in range(2):
                            nc.tensor.matmul(
                                st[:, half * 512:(half + 1) * 512],
                                kt_sb[h * HD:(h + 1) * HD,
                                      k0 + half * 128:k0 + (half + 1) * 128],
                                qt_sb[h * HD:(h + 1) * HD, q0:q0 + 512],
                                start=True, stop=True,
                            )
                        pt = ptp.tile([128, 1024], dt.bfloat16, tag="pt",
                                      name=f"pt_{b}_{qb}_{jp}_{h}")
                        nc.scalar.activation(pt[:], st[:], Exp, scale=0.125)
                        pts[(jp, h)] = pt

                def emit_pv(jp):
                    g0 = b * NJP * 2 + jp * 2
                    for h in range(NH):
                        pt = pts.pop((jp, h))
                        for half in range(2):
                            g = g0 + half
                            va = va_sb[:, g * NH * VW + h * VW:
                                       g * NH * VW + (h + 1) * VW]
                            nc.tensor.matmul(
                                pvs[h][:], va[:],
                                pt[:, half * 512:(half + 1) * 512],
                                start=(jp == 0 and half == 0),
                                stop=(jp == NJP - 1 and half == 1),
                            )

                for jp in range(NJP):
                    emit_st(jp)
                    fill(1)
                    if jp > 1:
                        emit_pv(jp - 2)
                emit_pv(NJP - 2)
                fill(1)
                emit_pv(NJP - 1)
                if last:
                    # last q-block: skip the normalize broadcast chain — copy
                    # UNNORMALIZED O^T straight to ot and extract token-major
                    # reciprocals [128, 4] (token t of subtile s at partition
                    # t, column s); the tail's per-head output projection
                    # applies them as per-partition drain scales
                    r4bs = {}
                    for h in range(NH):
                        osl = ot_sb[h * HD:(h + 1) * HD, q0:q0 + 512]
                        nc.vector.tensor_copy(osl, pvs[h][0:HD, :])
                        m64 = npl.tile([VW, 512], dt.float32, tag="m64",
                                       name=f"m64_{h}")
                        nc.vector.tensor_copy(m64[HD:VW, :], pvs[h][HD:VW, :])
                        d4b = npl.tile([128, 4], dt.float32, tag="d4b",
                                       name=f"d4b_{h}")
                        engs = [nc.sync, nc.scalar, nc.gpsimd]
                        for s in range(4):
                            engs[(h * 4 + s) % 3].dma_start(
                                d4b[:, s:s + 1],
                                m64[HD:VW, s * 128:(s + 1) * 128],
                            )
                        r4b = npl.tile([128, 4], dt.float32, tag="r4b",
                                       name=f"r4b_{h}")
                        nc.vector.reciprocal(r4b[:], d4b[:])
                        r4bs[h] = r4b
                    fill(100)
                    return r4bs
                # copy both heads' PV to SBUF first (frees PSUM), launch the
                # denominator-reshape DMAs immediately after each copy, and
                # only then run the reciprocals: by the time the in-order DVE
                # queue reaches recip h0, its d4 DMA has landed — the DVE
                # must never sit waiting on a DMA, since that head-of-line
                # blocking stalls the PSUM-recycling casts the PE filler
                # matmuls depend on
                ms = {}
                d4s = {}
                for h in range(NH):
                    m = npl.tile([VW, 512], dt.float32, tag="m", name=f"m_{b}_{qb}_{h}")
                    nc.vector.tensor_copy(m[:], pvs[h][:])
                    d4 = npl.tile([128, 4], dt.float32, tag="d4",
                                  name=f"d4_{b}_{qb}_{h}")
                    nc.sync.dma_start(
                        d4[:],
                        m[HD:VW, :].rearrange("p (a c) -> p a c", c=4),
                    )
                    ms[h] = m
                    d4s[h] = d4
                fill(2)
                for h in range(NH):
                    m = ms[h]
                    r4 = npl.tile([128, 4], dt.float32, tag="r4", name=f"r4_{b}_{qb}_{h}")
                    nc.vector.reciprocal(r4[:], d4s[h][:])
                    rc = npl.tile([1, 512], dt.float32, tag="rc", name=f"rc_{b}_{qb}_{h}")
                    nc.sync.dma_start(
                        rc[:].rearrange("p (a c) -> p a c", c=4), r4[:]
                    )
                    rbt = npl.tile([HD, 512], dt.float32, tag="rb", name=f"rb_{b}_{qb}_{h}")
                    nc.sync.dma_start(
                        rbt[:],
                        rc[0:1, :].rearrange("p (o q) -> p o q", o=1)
                        .broadcast_to((1, HD, 512)),
                    )
                    # bv is folded into the host-side gather (bv @ Wo^T is
                    # a constant row vector), so the normalize is a single
                    # mul. GpSimd mid-kernel (keeps the DVE queue unblocked);
                    # DVE in the last q-blocks, where the GpSimd wake-up
                    # latency would sit on the tail critical path.
                    osl = ot_sb[h * HD:(h + 1) * HD, q0:q0 + 512]
                    eng = nc.vector if late else nc.gpsimd
                    eng.tensor_mul(osl, m[0:HD, :], rbt[:])
                fill(100)   # drain any leftover fillers

            def p3_tile_scaled(g, r4bs):
                """Per-head output projection for the last q-block: the two
                64-contraction matmuls pair concurrently on the PE-array
                halves, and the softmax reciprocal is applied per-token as a
                drain scale (ACT for head 0, DVE mult-add for head 1)."""
                s = g % 4
                o_sb = op.tile([128, EMB], dt.float16, tag="osb", name=f"o_{g}")
                for ch in range(2):
                    ps0 = ps.tile([128, 512], dt.float32, tag="pp", bufs=2,
                                  name=f"opsa_{g}_{ch}")
                    ps1 = ps.tile([128, 512], dt.float32, tag="pp", bufs=2,
                                  name=f"opsb_{g}_{ch}")
                    nc.tensor.matmul(
                        ps0[:], ot_sb[0:HD, g * 128:(g + 1) * 128],
                        wo_sb[0:HD, ch * 512:(ch + 1) * 512],
                        start=True, stop=True,
                    )
                    nc.tensor.matmul(
                        ps1[:], ot_sb[HD:F, g * 128:(g + 1) * 128],
                        wo_sb[HD:F, ch * 512:(ch + 1) * 512],
                        start=True, stop=True,
                    )
                    dst = o_sb[:, ch * 512:(ch + 1) * 512]
                    nc.scalar.activation(
                        dst, ps0[:], mybir.ActivationFunctionType.Copy,
                        scale=r4bs[0][:, s:s + 1],
                    )
                    nc.vector.scalar_tensor_tensor(
                        dst, ps1[:], r4bs[1][:, s:s + 1], dst,
                        op0=mybir.AluOpType.mult, op1=mybir.AluOpType.add,
                    )
                nc.sync.dma_start(out_ext[g * 128:(g + 1) * 128, :], o_sb[:])

            # ---------------- emission schedule ----------------
            # prologue: first 3 projection blocks upfront
            for rb in range(PROLOGUE_BLOCKS):
                for f in p1_block_fillers(rb):
                    f()
            # p3 tiles per q-block: lag >= 2 q-blocks behind (so ot rows are
            # long past the normalize chain); back-load the distribution so
            # the last batch — which has no projection fillers left — still
            # keeps the PE busy.
            P3_COUNT = [0, 0, 3, 3, 3, 3, 3, 3, 3, 3, 3, 3, 7, 7, 6, 6]
            nxt_g = 0
            for b in range(B):
                for qb in range(NQB):
                    i = b * NQB + qb          # global q-block index 0..15
                    fillers = []
                    blk = PROLOGUE_BLOCKS + i  # p1 block for this q-block
                    if blk < NBLK:
                        fillers.extend(p1_block_fillers(blk))
                    hi = min(nxt_g + P3_COUNT[i], (i - 1) * 4 if i > 1 else 0)
                    for g in range(nxt_g, hi):
                        fillers.append(lambda g=g: p3_tile(g, act_drain=False))
                    nxt_g = max(nxt_g, hi)
                    r4bs = p2_qblock(b, qb, fillers,
                                     late=(i >= NQB * B - 2),
                                     last=(i == NQB * B - 1))
            # tail: normal projection for q-block 14's tiles, scaled per-head
            # projection for q-block 15's (ot unnormalized there)
            for g in range(nxt_g, G - 4):
                p3_tile(g, act_drain=True)
            for g in range(G - 4, G):
                p3_tile_scaled(g, r4bs)
    return nc


_NC_CACHE = None


def _get_nc():
    global _NC_CACHE
    if _NC_CACHE is None:
        _NC_CACHE = build_bass()
    return _NC_CACHE


def make_in_maps(x, Wq, bq, Wk, bk, Wv, bv, Wo, bo):
    xt = np.ascontiguousarray(
        np.asarray(x, dtype=np.float32).reshape(R, EMB).astype(bf16).T
    )
    in_maps = []
    for c in range(NCORES):
        rows = slice(F * c, F * (c + 1))
        in_maps.append({
            "xt": xt,
            "wq": np.ascontiguousarray(np.asarray(Wq)[rows, :].T.astype(bf16)),
            "wk": np.ascontiguousarray(np.asarray(Wk)[rows, :].T.astype(bf16)),
            "wv": np.ascontiguousarray(np.asarray(Wv)[rows, :].T.astype(bf16)),
            "wo": np.ascontiguousarray(np.asarray(Wo)[:, rows].T.astype(bf16)),
            "bq": np.asarray(bq)[rows].reshape(F, 1).astype(np.float32),
        })
    return in_maps


def gather(results, bv, Wo, bo):
    acc = np.zeros((R, EMB), np.float32)
    for r in results:
        acc += r["out"].astype(np.float32)
    # bv rides through the output projection as a constant row vector
    acc += np.asarray(bo, dtype=np.float32) + (
        np.asarray(bv, dtype=np.float64) @ np.asarray(Wo, dtype=np.float64).T
    ).astype(np.float32)
    return acc.reshape(B, T, EMB)


def kernel(x, Wq, bq, Wk, bk, Wv, bv, Wo, bo, _trace=False):
    nc = _get_nc()
    in_maps = make_in_maps(x, Wq, bq, Wk, bk, Wv, bv, Wo, bo)
    res = run_bass_kernel_spmd(nc, in_maps, list(range(NCORES)), trace=_trace)
    out = gather(res.results, bv, Wo, bo)
    if _trace:
        kernel.last_result = res
    return out


# revision 39
# speedup vs baseline: 1.0460x; 1.0072x over previous
"""Multi-head self-attention (B=4, T=2048, C=1024, 16 heads x hd=64) on 8
Trainium2 NeuronCores.

Sharding: tensor-parallel over heads — each core owns 2 heads (128 of the
1024 channels): its slices of Wq/Wk/Wv rows and Wo columns. Every core reads
the full x (transposed + bf16-cast on host), computes Q^T/K^T (channel-major)
and V (token-major via PE transpose) for its heads, runs attention entirely
from SBUF, then produces a rank-128 partial of the output projection. The 8
partials are summed on host (+ bo).

Per-core dataflow (all matmuls bf16 in / fp32 PSUM accumulate):
  phase 1: per 512-row block: Q^T = Wq_c @ x^T (+bq), K^T = Wk_c @ x^T (bk
           dropped — its score term q.bk is constant along the softmax
           axis), V = x @ Wv_c^T token-major with a ones column appended
           per head (denominator trick). All SBUF-resident.
  phase 2: per (batch, 512-query block): S^T [128k, 1024(2 k-tiles)] per head
           via K^T-stationary matmuls (contraction d=64), one exp per k-tile
           pair on ScalarE (scale=1/8 folded in) -> P^T bf16, then
           O^T[65,512] += [V|1]^T P^T accumulated over k with K=128 matmuls.
           Softmax denominator lands in row 64; its reciprocal is taken
           single-lane on VectorE, broadcast down 64 partitions with one
           zero-stride DMA; the normalize mul runs on GpSimd (bv rides
           through Wo on the host side as a constant row vector).
  phase 3: partial_out[128 rows, 512] = O^T-slice-stationary matmuls against
           Wo_c^T; fp16 partials DMA'd out.

Schedule: 3 projection blocks upfront; remaining 13 blocks are fillers for
global q-blocks 0..12; each q-block also drains the output projection of
the q-block TWO back (so its ot rows are long past the normalize chain),
leaving only the last two q-blocks' projections for the tail.
"""
import json

import numpy as np
import ml_dtypes

import concourse.bass as bass
import concourse.mybir as mybir
import concourse.tile as tile
from concourse.bass_utils import run_bass_kernel_spmd

bf16 = ml_dtypes.bfloat16
dt = mybir.dt

EMB = 1024
HEADS = 16
HD = 64
B = 4
T = 2048
R = B * T            # 8192 rows
NCORES = 8
F = EMB // NCORES    # 128 channels (2 heads) per core
NH = F // HD         # 2 heads per core
NKC = EMB // 128     # 8 contraction chunks for projections
NQB = T // 512       # 4 query blocks per batch
NJP = T // 256       # 8 k-tile PAIRS per batch
G = R // 128         # 64 global row/key tiles
VW = HD + 1          # 65: V head slice + ones column
NBLK = R // 512      # 16 projection row-blocks
PROLOGUE_BLOCKS = 3


# ---------------------------------------------------------------------------
# walrus in this container accepts only ONE sync-wait per instruction; split
# extra waits onto same-engine NoOps at BIR-serialization time.
_orig_to_json_bytes = bass.Bass.to_json_bytes


def _split_waits(data: bytes) -> bytes:
    d = json.loads(data)
    changed = False
    for f in d.get("functions", []):
        for blk in f.get("blocks", []):
            out = []
            for inst in blk.get("instructions", []):
                si = inst.get("sync_info")
                waits = (si or {}).get("on_wait") or []
                if len(waits) > 1:
                    changed = True
                    for i, w in enumerate(waits[:-1]):
                        out.append({
                            "debug": inst.get("debug", 0),
                            "engine": inst["engine"],
                            "ins": [], "outs": [],
                            "name": f"{inst['name']}_w{i}",
                            "opcode": "NoOp",
                            "sync_info": {"on_update": [], "on_wait": [w]},
                            "text_hint": "wait_split",
                        })
                    si["on_wait"] = waits[-1:]
                out.append(inst)
            blk["instructions"] = out
    return json.dumps(d).encode() if changed else data


def _to_json_bytes(self, *a, **k):
    return _split_waits(_orig_to_json_bytes(self, *a, **k))


bass.Bass.to_json_bytes = _to_json_bytes
# ---------------------------------------------------------------------------


def build_bass() -> bass.Bass:
    nc = bass.Bass()
    xt_ext = nc.declare_dram_parameter("xt", [EMB, R], dt.bfloat16, isOutput=False)
    wq_ext = nc.declare_dram_parameter("wq", [EMB, F], dt.bfloat16, isOutput=False)
    wk_ext = nc.declare_dram_parameter("wk", [EMB, F], dt.bfloat16, isOutput=False)
    wv_ext = nc.declare_dram_parameter("wv", [EMB, F], dt.bfloat16, isOutput=False)
    wo_ext = nc.declare_dram_parameter("wo", [F, EMB], dt.bfloat16, isOutput=False)
    bq_ext = nc.declare_dram_parameter("bq", [F, 1], dt.float32, isOutput=False)
    out_ext = nc.declare_dram_parameter("out", [R, EMB], dt.float16, isOutput=True)

    Exp = mybir.ActivationFunctionType.Exp

    with tile.TileContext(nc) as tc:
        with (
            tc.tile_pool(name="const", bufs=1) as cp,
            tc.tile_pool(name="res", bufs=1) as res,
            tc.tile_pool(name="xt", bufs=2) as xp,
            tc.tile_pool(name="pt", bufs=6) as ptp,
            tc.tile_pool(name="norm", bufs=2) as npl,
            tc.tile_pool(name="osb", bufs=3) as op,
            tc.tile_pool(name="ps", bufs=1, space="PSUM") as ps,
        ):
            # --- constants ---
            wq_sb = cp.tile([128, EMB], dt.bfloat16, tag="wq")
            wk_sb = cp.tile([128, EMB], dt.bfloat16, tag="wk")
            wv_sb = cp.tile([128, EMB], dt.bfloat16, tag="wv")
            wo_sb = cp.tile([128, EMB], dt.bfloat16, tag="wo")
            bq_sb = cp.tile([F, 1], dt.float32, tag="bq")
            def _wload(ext, tile_sb):
                nc.sync.dma_start(
                    tile_sb[:].rearrange("p (kc f) -> p kc f", f=F),
                    ext[:].rearrange("(kc p) f -> p kc f", p=128),
                )
            _wload(wq_ext, wq_sb)
            nc.sync.dma_start(bq_sb[:], bq_ext[:])
            _wload(wk_ext, wk_sb)
            _wload(wv_ext, wv_sb)
            nc.sync.dma_start(wo_sb[:], wo_ext[:])

            # --- residents ---
            qt_sb = res.tile([F, R], dt.bfloat16, tag="qt")
            kt_sb = res.tile([F, R], dt.bfloat16, tag="kt")
            ot_sb = res.tile([F, R], dt.bfloat16, tag="ot")
            va_sb = res.tile([128, G * NH * VW], dt.bfloat16, tag="va")
            nc.vector.memset(
                va_sb[:].rearrange("p (g d) -> p g d", d=VW)[:, :, HD:VW], 1.0
            )

            # ---- phase-1 emitters (one 512-row block = 5 filler units) ----
            def p1_load(rb, eng=None):
                xts = []
                for kc in range(NKC):
                    xt = xp.tile([128, 512], dt.bfloat16, tag=f"xt{kc}",
                                 name=f"xt{kc}_{rb}")
                    (eng or nc.sync).dma_start(
                        xt[:],
                        xt_ext[kc * 128:(kc + 1) * 128, rb * 512:rb * 512 + 512],
                    )
                    xts.append(xt)
                return xts

            def p1_qk(rb, xts, w_sb, dst_sb, bias):
                r0 = rb * 512
                acc = ps.tile([128, 512], dt.float32, tag="pp", bufs=2,
                              name=f"prj_{rb}_{id(w_sb)}")
                for kc in range(NKC):
                    nc.tensor.matmul(
                        acc[:], w_sb[:, kc * F:(kc + 1) * F], xts[kc][:],
                        start=(kc == 0), stop=(kc == NKC - 1),
                    )
                if bias is not None:
                    nc.vector.tensor_scalar_add(dst_sb[:, r0:r0 + 512], acc[:], bias[:])
                else:
                    nc.vector.tensor_copy(dst_sb[:, r0:r0 + 512], acc[:])

            def p1_v(rb, xts, sub):
                g = rb * 4 + sub
                acc = ps.tile([128, F], dt.float32, tag="pp", bufs=2,
                              name=f"vprj_{g}")
                for kc in range(NKC):
                    nc.tensor.matmul(
                        acc[:],
                        xts[kc][:, sub * 128:(sub + 1) * 128],
                        wv_sb[:, kc * F:(kc + 1) * F],
                        start=(kc == 0), stop=(kc == NKC - 1),
                    )
                dst = va_sb[:, g * NH * VW:(g + 1) * NH * VW].rearrange(
                    "p (h d) -> p h d", d=VW
                )[:, :, 0:HD]
                nc.vector.tensor_copy(
                    dst, acc[:].rearrange("p (h d) -> p h d", d=HD)
                )

            def p1_block_fillers(rb):
                """7 filler closures for one 512-row projection block."""
                state = {}

                def load():
                    state["xts"] = p1_load(rb)

                fillers = [load,
                           lambda: p1_qk(rb, state["xts"], wq_sb, qt_sb, bq_sb),
                           lambda: p1_qk(rb, state["xts"], wk_sb, kt_sb, None)]
                for sub in range(4):
                    fillers.append(lambda s=sub: p1_v(rb, state["xts"], s))
                return fillers

            # ---- phase-3 emitter (one 128-row tile) ----
            def p3_tile(g, act_drain=True, out_eng=None):
                """Drain one PSUM half on the ACT engine (Copy is in every
                table set — no reload) so the two halves' drains run in
                parallel and the DVE queue stays short. Skipped in the late
                q-blocks, where the denser p3 schedule would overload ACT
                next to its exp stream."""
                o_sb = op.tile([128, EMB], dt.float16, tag="osb", name=f"o_{g}")
                for ch in range(2):
                    o_ps = ps.tile([128, 512], dt.float32, tag="pp", bufs=2,
                                   name=f"ops_{g}_{ch}")
                    nc.tensor.matmul(
                        o_ps[:],
                        ot_sb[:, g * 128:(g + 1) * 128],
                        wo_sb[:, ch * 512:(ch + 1) * 512],
                        start=True, stop=True,
                    )
                    dst = o_sb[:, ch * 512:(ch + 1) * 512]
                    if ch == 1 and act_drain:
                        nc.scalar.activation(
                            dst, o_ps[:], mybir.ActivationFunctionType.Copy
                        )
                    else:
                        nc.vector.tensor_copy(dst, o_ps[:])
                (out_eng or nc.sync).dma_start(
                    out_ext[g * 128:(g + 1) * 128, :], o_sb[:])

            # ---- phase-2 q-block with interleaved fillers ----
            def p2_qblock(b, qb, fillers, late=False, last=False):
                q0 = b * T + qb * 512
                fi = iter(fillers)

                def fill(n=1):
                    for _ in range(n):
                        f = next(fi, None)
                        if f is not None:
                            f()

                pvs = {h: ps.tile([VW, 512], dt.float32, tag="pv", bufs=2,
                                  name=f"pv_{b}_{qb}_{h}")
                       for h in range(NH)}
                pts = {}

                def emit_st(jp):
                    k0 = b * T + jp * 256
                    for h in range(NH):
                        st = ps.tile([128, 1024], dt.float32, tag="st", bufs=2,
                                     name=f"st_{b}_{qb}_{jp}_{h}")
                        for half in range(2):
                            nc.tensor.matmul(
                                st[:, half * 512:(half + 1) * 512],
                                kt_sb[h * HD:(h + 1) * HD,
                                      k0 + half * 128:k0 + (half + 1) * 128],
                                qt_sb[h * HD:(h + 1) * HD, q0:q0 + 512],
                                start=True, stop=True,
                            )
                        pt = ptp.tile([128, 1024], dt.bfloat16, tag="pt",
                                      name=f"pt_{b}_{qb}_{jp}_{h}")
                        nc.scalar.activation(pt[:], st[:], Exp, scale=0.125)
                        pts[(jp, h)] = pt

                def emit_pv(jp):
                    g0 = b * NJP * 2 + jp * 2
                    for h in range(NH):
                        pt = pts.pop((jp, h))
                        for half in range(2):
                            g = g0 + half
                            va = va_sb[:, g * NH * VW + h * VW:
                                       g * NH * VW + (h + 1) * VW]
                            nc.tensor.matmul(
                                pvs[h][:], va[:],
                                pt[:, half * 512:(half + 1) * 512],
                                start=(jp == 0 and half == 0),
                                stop=(jp == NJP - 1 and half == 1),
                            )

                for jp in range(NJP):
                    emit_st(jp)
                    fill(1)
                    if jp > 1:
                        emit_pv(jp - 2)
                emit_pv(NJP - 2)
                fill(1)
                emit_pv(NJP - 1)
                if last:
                    # last q-block: skip the normalize broadcast chain — copy
                    # UNNORMALIZED O^T straight to ot and extract token-major
                    # reciprocals [128, 4] (token t of subtile s at partition
                    # t, column s); the tail's per-head output projection
                    # applies them as per-partition drain scales
                    r4bs = {}
                    for h in range(NH):
                        osl = ot_sb[h * HD:(h + 1) * HD, q0:q0 + 512]
                        nc.vector.tensor_copy(osl, pvs[h][0:HD, :])
                        m64 = npl.tile([VW, 512], dt.float32, tag="m64",
                                       bufs=4, name=f"m64_{qb}_{h}")
                        nc.vector.tensor_copy(m64[HD:VW, :], pvs[h][HD:VW, :])
                        d4b = npl.tile([128, 4], dt.float32, tag="d4b",
                                       bufs=4, name=f"d4b_{qb}_{h}")
                        engs = [nc.sync, nc.scalar, nc.gpsimd]
                        for s in range(4):
                            engs[(h * 4 + s) % 3].dma_start(
                                d4b[:, s:s + 1],
                                m64[HD:VW, s * 128:(s + 1) * 128],
                            )
                        r4b = npl.tile([128, 4], dt.float32, tag="r4b",
                                       bufs=4, name=f"r4b_{qb}_{h}")
                        nc.vector.reciprocal(r4b[:], d4b[:])
                        r4bs[h] = r4b
                    fill(100)
                    return r4bs
                # copy both heads' PV to SBUF first (frees PSUM), launch the
                # denominator-reshape DMAs immediately after each copy, and
                # only then run the reciprocals: by the time the in-order DVE
                # queue reaches recip h0, its d4 DMA has landed — the DVE
                # must never sit waiting on a DMA, since that head-of-line
                # blocking stalls the PSUM-recycling casts the PE filler
                # matmuls depend on
                ms = {}
                d4s = {}
                for h in range(NH):
                    m = npl.tile([VW, 512], dt.float32, tag="m", name=f"m_{b}_{qb}_{h}")
                    nc.vector.tensor_copy(m[:], pvs[h][:])
                    d4 = npl.tile([128, 4], dt.float32, tag="d4",
                                  name=f"d4_{b}_{qb}_{h}")
                    nc.sync.dma_start(
                        d4[:],
                        m[HD:VW, :].rearrange("p (a c) -> p a c", c=4),
                    )
                    ms[h] = m
                    d4s[h] = d4
                fill(2)
                for h in range(NH):
                    m = ms[h]
                    r4 = npl.tile([128, 4], dt.float32, tag="r4", name=f"r4_{b}_{qb}_{h}")
                    nc.vector.reciprocal(r4[:], d4s[h][:])
                    rc = npl.tile([1, 512], dt.float32, tag="rc", name=f"rc_{b}_{qb}_{h}")
                    nc.sync.dma_start(
                        rc[:].rearrange("p (a c) -> p a c", c=4), r4[:]
                    )
                    rbt = npl.tile([HD, 512], dt.float32, tag="rb", name=f"rb_{b}_{qb}_{h}")
                    nc.sync.dma_start(
                        rbt[:],
                        rc[0:1, :].rearrange("p (o q) -> p o q", o=1)
                        .broadcast_to((1, HD, 512)),
                    )
                    # bv is folded into the host-side gather (bv @ Wo^T is
                    # a constant row vector), so the normalize is a single
                    # mul. GpSimd mid-kernel (keeps the DVE queue unblocked);
                    # DVE in the last q-blocks, where the GpSimd wake-up
                    # latency would sit on the tail critical path.
                    osl = ot_sb[h * HD:(h + 1) * HD, q0:q0 + 512]
                    eng = nc.vector if late else nc.gpsimd
                    eng.tensor_mul(osl, m[0:HD, :], rbt[:])
                fill(100)   # drain any leftover fillers

            def p3_tile_scaled(g, r4bs):
                """Per-head output projection for the last q-block: the two
                64-contraction matmuls pair concurrently on the PE-array
                halves, and the softmax reciprocal is applied per-token as a
                drain scale (ACT for head 0, DVE mult-add for head 1)."""
                s = g % 4
                o_sb = op.tile([128, EMB], dt.float16, tag="osb", name=f"o_{g}")
                for ch in range(2):
                    ps0 = ps.tile([128, 512], dt.float32, tag="pp", bufs=2,
                                  name=f"opsa_{g}_{ch}")
                    ps1 = ps.tile([128, 512], dt.float32, tag="pp", bufs=2,
                                  name=f"opsb_{g}_{ch}")
                    nc.tensor.matmul(
                        ps0[:], ot_sb[0:HD, g * 128:(g + 1) * 128],
                        wo_sb[0:HD, ch * 512:(ch + 1) * 512],
                        start=True, stop=True,
                    )
                    nc.tensor.matmul(
                        ps1[:], ot_sb[HD:F, g * 128:(g + 1) * 128],
                        wo_sb[HD:F, ch * 512:(ch + 1) * 512],
                        start=True, stop=True,
                    )
                    dst = o_sb[:, ch * 512:(ch + 1) * 512]
                    nc.scalar.activation(
                        dst, ps0[:], mybir.ActivationFunctionType.Copy,
                        scale=r4bs[0][:, s:s + 1],
                    )
                    nc.vector.scalar_tensor_tensor(
                        dst, ps1[:], r4bs[1][:, s:s + 1], dst,
                        op0=mybir.AluOpType.mult, op1=mybir.AluOpType.add,
                    )
                nc.sync.dma_start(out_ext[g * 128:(g + 1) * 128, :], o_sb[:])

            # ---------------- emission schedule ----------------
            # prologue: first 3 projection blocks upfront
            for rb in range(PROLOGUE_BLOCKS):
                for f in p1_block_fillers(rb):
                    f()
            # p3 tiles per q-block: lag >= 2 q-blocks behind (so ot rows are
            # long past the normalize chain); back-load the distribution so
            # the last batch — which has no projection fillers left — still
            # keeps the PE busy.
            P3_COUNT = [0, 0, 3, 3, 3, 3, 3, 3, 3, 3, 3, 3, 7, 7, 6, 6]
            nxt_g = 0
            for b in range(B):
                for qb in range(NQB):
                    i = b * NQB + qb          # global q-block index 0..15
                    fillers = []
                    blk = PROLOGUE_BLOCKS + i  # p1 block for this q-block
                    if blk < NBLK:
                        fillers.extend(p1_block_fillers(blk))
                    hi = min(nxt_g + P3_COUNT[i], (i - 1) * 4 if i > 1 else 0)
                    for g in range(nxt_g, hi):
                        fillers.append(lambda g=g: p3_tile(g, act_drain=False))
                    nxt_g = max(nxt_g, hi)
                    r = p2_qblock(b, qb, fillers,
                                  late=(i >= NQB * B - 2),
                                  last=(i >= NQB * B - 2))
                    if r is not None:
                        r4bs_by_qb = locals().get('r4bs_by_qb') or {}
                        r4bs_by_qb[i] = r
            # tail: scaled per-head projection for the last two q-blocks
            # (their ot rows are unnormalized; the reciprocal rides the
            # drain as a per-token scale)
            for g in range(nxt_g, G):
                p3_tile_scaled(g, r4bs_by_qb[14 + (g >= G - 4)])
    return nc


_NC_CACHE = None


def _get_nc():
    global _NC_CACHE
    if _NC_CACHE is None:
        _NC_CACHE = build_bass()
    return _NC_CACHE


def make_in_maps(x, Wq, bq, Wk, bk, Wv, bv, Wo, bo):
    xt = np.ascontiguousarray(
        np.asarray(x, dtype=np.float32).reshape(R, EMB).astype(bf16).T
    )
    in_maps = []
    for c in range(NCORES):
        rows = slice(F * c, F * (c + 1))
        in_maps.append({
            "xt": xt,
            "wq": np.ascontiguousarray(np.asarray(Wq)[rows, :].T.astype(bf16)),
            "wk": np.ascontiguousarray(np.asarray(Wk)[rows, :].T.astype(bf16)),
            "wv": np.ascontiguousarray(np.asarray(Wv)[rows, :].T.astype(bf16)),
            "wo": np.ascontiguousarray(np.asarray(Wo)[:, rows].T.astype(bf16)),
            "bq": np.asarray(bq)[rows].reshape(F, 1).astype(np.float32),
        })
    return in_maps


def gather(results, bv, Wo, bo):
    acc = np.zeros((R, EMB), np.float32)
    for r in results:
        acc += r["out"].astype(np.float32)
    # bv rides through the output projection as a constant row vector
    acc += np.asarray(bo, dtype=np.float32) + (
        np.asarray(bv, dtype=np.float64) @ np.asarray(Wo, dtype=np.float64).T
    ).astype(np.float32)
    return acc.reshape(B, T, EMB)


def kernel(x, Wq, bq, Wk, bk, Wv, bv, Wo, bo, _trace=False):
    nc = _get_nc()
    in_maps = make_in_maps(x, Wq, bq, Wk, bk, Wv, bv, Wo, bo)
    res = run_bass_kernel_spmd(nc, in_maps, list(range(NCORES)), trace=_trace)
    out = gather(res.results, bv, Wo, bo)
    if _trace:
        kernel.last_result = res
    return out


# revision 40
# speedup vs baseline: 1.0674x; 1.0205x over previous
"""Multi-head self-attention (B=4, T=2048, C=1024, 16 heads x hd=64) on 8
Trainium2 NeuronCores.

Sharding: tensor-parallel over heads — each core owns 2 heads (128 of the
1024 channels): its slices of Wq/Wk/Wv rows and Wo columns. Every core reads
the full x (transposed + bf16-cast on host), computes Q^T/K^T (channel-major)
and V (token-major via PE transpose) for its heads, runs attention entirely
from SBUF, then produces a rank-128 partial of the output projection. The 8
partials are summed on host (+ bo).

Per-core dataflow (all matmuls bf16 in / fp32 PSUM accumulate):
  phase 1: per 512-row block: Q^T = Wq_c @ x^T (+bq), K^T = Wk_c @ x^T (bk
           dropped — its score term q.bk is constant along the softmax
           axis), V = x @ Wv_c^T token-major with a ones column appended
           per head (denominator trick). All SBUF-resident.
  phase 2: per (batch, 512-query block): S^T [128k, 1024(2 k-tiles)] per head
           via K^T-stationary matmuls (contraction d=64), one exp per k-tile
           pair on ScalarE (scale=1/8 folded in) -> P^T bf16, then
           O^T[65,512] += [V|1]^T P^T accumulated over k with K=128 matmuls.
           Softmax denominator lands in row 64; its reciprocal is taken
           single-lane on VectorE, broadcast down 64 partitions with one
           zero-stride DMA; the normalize mul runs on GpSimd (bv rides
           through Wo on the host side as a constant row vector).
  phase 3: partial_out[128 rows, 512] = O^T-slice-stationary matmuls against
           Wo_c^T; fp16 partials DMA'd out.

Schedule: 3 projection blocks upfront; remaining 13 blocks are fillers for
global q-blocks 0..12; each q-block also drains the output projection of
the q-block TWO back (so its ot rows are long past the normalize chain),
leaving only the last two q-blocks' projections for the tail.
"""
import json

import numpy as np
import ml_dtypes

import concourse.bass as bass
import concourse.mybir as mybir
import concourse.tile as tile
from concourse.bass_utils import run_bass_kernel_spmd

bf16 = ml_dtypes.bfloat16
dt = mybir.dt

EMB = 1024
HEADS = 16
HD = 64
B = 4
T = 2048
R = B * T            # 8192 rows
NCORES = 8
F = EMB // NCORES    # 128 channels (2 heads) per core
NH = F // HD         # 2 heads per core
NKC = EMB // 128     # 8 contraction chunks for projections
NQB = T // 512       # 4 query blocks per batch
NJP = T // 256       # 8 k-tile PAIRS per batch
G = R // 128         # 64 global row/key tiles
VW = HD + 1          # 65: V head slice + ones column
NBLK = R // 512      # 16 projection row-blocks
PROLOGUE_BLOCKS = 3


# ---------------------------------------------------------------------------
# walrus in this container accepts only ONE sync-wait per instruction; split
# extra waits onto same-engine NoOps at BIR-serialization time.
_orig_to_json_bytes = bass.Bass.to_json_bytes


def _split_waits(data: bytes) -> bytes:
    d = json.loads(data)
    changed = False
    for f in d.get("functions", []):
        for blk in f.get("blocks", []):
            out = []
            for inst in blk.get("instructions", []):
                si = inst.get("sync_info")
                waits = (si or {}).get("on_wait") or []
                if len(waits) > 1:
                    changed = True
                    for i, w in enumerate(waits[:-1]):
                        out.append({
                            "debug": inst.get("debug", 0),
                            "engine": inst["engine"],
                            "ins": [], "outs": [],
                            "name": f"{inst['name']}_w{i}",
                            "opcode": "NoOp",
                            "sync_info": {"on_update": [], "on_wait": [w]},
                            "text_hint": "wait_split",
                        })
                    si["on_wait"] = waits[-1:]
                out.append(inst)
            blk["instructions"] = out
    return json.dumps(d).encode() if changed else data


def _to_json_bytes(self, *a, **k):
    return _split_waits(_orig_to_json_bytes(self, *a, **k))


bass.Bass.to_json_bytes = _to_json_bytes
# ---------------------------------------------------------------------------


def build_bass() -> bass.Bass:
    nc = bass.Bass()
    xt_ext = nc.declare_dram_parameter("xt", [EMB, R], dt.bfloat16, isOutput=False)
    wq_ext = nc.declare_dram_parameter("wq", [EMB, F], dt.bfloat16, isOutput=False)
    wk_ext = nc.declare_dram_parameter("wk", [EMB, F], dt.bfloat16, isOutput=False)
    wv_ext = nc.declare_dram_parameter("wv", [EMB, F], dt.bfloat16, isOutput=False)
    wo_ext = nc.declare_dram_parameter("wo", [F, EMB], dt.bfloat16, isOutput=False)
    bq_ext = nc.declare_dram_parameter("bq", [F, 1], dt.float32, isOutput=False)
    out_ext = nc.declare_dram_parameter("out", [R, EMB], dt.float16, isOutput=True)

    Exp = mybir.ActivationFunctionType.Exp

    with tile.TileContext(nc) as tc:
        with (
            tc.tile_pool(name="const", bufs=1) as cp,
            tc.tile_pool(name="res", bufs=1) as res,
            tc.tile_pool(name="xt", bufs=2) as xp,
            tc.tile_pool(name="pt", bufs=6) as ptp,
            tc.tile_pool(name="norm", bufs=2) as npl,
            tc.tile_pool(name="osb", bufs=3) as op,
            tc.tile_pool(name="ps", bufs=1, space="PSUM") as ps,
        ):
            # --- constants ---
            wq_sb = cp.tile([128, EMB], dt.bfloat16, tag="wq")
            wk_sb = cp.tile([128, EMB], dt.bfloat16, tag="wk")
            wv_sb = cp.tile([128, EMB], dt.bfloat16, tag="wv")
            wo_sb = cp.tile([128, EMB], dt.bfloat16, tag="wo")
            bq_sb = cp.tile([F, 1], dt.float32, tag="bq")
            def _wload(ext, tile_sb):
                nc.sync.dma_start(
                    tile_sb[:].rearrange("p (kc f) -> p kc f", f=F),
                    ext[:].rearrange("(kc p) f -> p kc f", p=128),
                )
            _wload(wq_ext, wq_sb)
            nc.sync.dma_start(bq_sb[:], bq_ext[:])
            _wload(wk_ext, wk_sb)
            _wload(wv_ext, wv_sb)
            nc.sync.dma_start(wo_sb[:], wo_ext[:])

            # --- residents ---
            qt_sb = res.tile([F, R], dt.bfloat16, tag="qt")
            kt_sb = res.tile([F, R], dt.bfloat16, tag="kt")
            ot_sb = res.tile([F, R], dt.bfloat16, tag="ot")
            va_sb = res.tile([128, G * NH * VW], dt.bfloat16, tag="va")
            nc.vector.memset(
                va_sb[:].rearrange("p (g d) -> p g d", d=VW)[:, :, HD:VW], 1.0
            )

            # ---- phase-1 emitters (one 512-row block = 5 filler units) ----
            def p1_load(rb, eng=None):
                xts = []
                for kc in range(NKC):
                    xt = xp.tile([128, 512], dt.bfloat16, tag=f"xt{kc}",
                                 name=f"xt{kc}_{rb}")
                    (eng or nc.sync).dma_start(
                        xt[:],
                        xt_ext[kc * 128:(kc + 1) * 128, rb * 512:rb * 512 + 512],
                    )
                    xts.append(xt)
                return xts

            def p1_qk(rb, xts, w_sb, dst_sb, bias):
                r0 = rb * 512
                acc = ps.tile([128, 512], dt.float32, tag="pp", bufs=2,
                              name=f"prj_{rb}_{id(w_sb)}")
                for kc in range(NKC):
                    nc.tensor.matmul(
                        acc[:], w_sb[:, kc * F:(kc + 1) * F], xts[kc][:],
                        start=(kc == 0), stop=(kc == NKC - 1),
                    )
                if bias is not None:
                    nc.vector.tensor_scalar_add(dst_sb[:, r0:r0 + 512], acc[:], bias[:])
                else:
                    nc.vector.tensor_copy(dst_sb[:, r0:r0 + 512], acc[:])

            def p1_v(rb, xts, sub):
                g = rb * 4 + sub
                acc = ps.tile([128, F], dt.float32, tag="pp", bufs=2,
                              name=f"vprj_{g}")
                for kc in range(NKC):
                    nc.tensor.matmul(
                        acc[:],
                        xts[kc][:, sub * 128:(sub + 1) * 128],
                        wv_sb[:, kc * F:(kc + 1) * F],
                        start=(kc == 0), stop=(kc == NKC - 1),
                    )
                dst = va_sb[:, g * NH * VW:(g + 1) * NH * VW].rearrange(
                    "p (h d) -> p h d", d=VW
                )[:, :, 0:HD]
                nc.vector.tensor_copy(
                    dst, acc[:].rearrange("p (h d) -> p h d", d=HD)
                )

            def p1_block_fillers(rb):
                """7 filler closures for one 512-row projection block."""
                state = {}

                def load():
                    state["xts"] = p1_load(rb)

                fillers = [load,
                           lambda: p1_qk(rb, state["xts"], wq_sb, qt_sb, bq_sb),
                           lambda: p1_qk(rb, state["xts"], wk_sb, kt_sb, None)]
                for sub in range(4):
                    fillers.append(lambda s=sub: p1_v(rb, state["xts"], s))
                return fillers

            # ---- phase-3 emitter (one 128-row tile) ----
            def p3_tile(g, act_drain=True, out_eng=None):
                """Drain one PSUM half on the ACT engine (Copy is in every
                table set — no reload) so the two halves' drains run in
                parallel and the DVE queue stays short. Skipped in the late
                q-blocks, where the denser p3 schedule would overload ACT
                next to its exp stream."""
                o_sb = op.tile([128, EMB], dt.float16, tag="osb", name=f"o_{g}")
                for ch in range(2):
                    o_ps = ps.tile([128, 512], dt.float32, tag="pp", bufs=2,
                                   name=f"ops_{g}_{ch}")
                    nc.tensor.matmul(
                        o_ps[:],
                        ot_sb[:, g * 128:(g + 1) * 128],
                        wo_sb[:, ch * 512:(ch + 1) * 512],
                        start=True, stop=True,
                    )
                    dst = o_sb[:, ch * 512:(ch + 1) * 512]
                    if ch == 1 and act_drain:
                        nc.scalar.activation(
                            dst, o_ps[:], mybir.ActivationFunctionType.Copy
                        )
                    else:
                        nc.vector.tensor_copy(dst, o_ps[:])
                (out_eng or nc.sync).dma_start(
                    out_ext[g * 128:(g + 1) * 128, :], o_sb[:])

            # ---- phase-2 q-block with interleaved fillers ----
            def p2_qblock(b, qb, fillers, late=False, last=False):
                q0 = b * T + qb * 512
                fi = iter(fillers)

                def fill(n=1):
                    for _ in range(n):
                        f = next(fi, None)
                        if f is not None:
                            f()

                pvs = {h: ps.tile([VW, 512], dt.float32, tag="pv", bufs=2,
                                  name=f"pv_{b}_{qb}_{h}")
                       for h in range(NH)}
                pts = {}

                def emit_st(jp):
                    k0 = b * T + jp * 256
                    for h in range(NH):
                        st = ps.tile([128, 1024], dt.float32, tag="st", bufs=2,
                                     name=f"st_{b}_{qb}_{jp}_{h}")
                        for half in range(2):
                            nc.tensor.matmul(
                                st[:, half * 512:(half + 1) * 512],
                                kt_sb[h * HD:(h + 1) * HD,
                                      k0 + half * 128:k0 + (half + 1) * 128],
                                qt_sb[h * HD:(h + 1) * HD, q0:q0 + 512],
                                start=True, stop=True,
                            )
                        pt = ptp.tile([128, 1024], dt.bfloat16, tag="pt",
                                      name=f"pt_{b}_{qb}_{jp}_{h}")
                        nc.scalar.activation(pt[:], st[:], Exp, scale=0.125)
                        pts[(jp, h)] = pt

                def emit_pv(jp):
                    g0 = b * NJP * 2 + jp * 2
                    for h in range(NH):
                        pt = pts.pop((jp, h))
                        for half in range(2):
                            g = g0 + half
                            va = va_sb[:, g * NH * VW + h * VW:
                                       g * NH * VW + (h + 1) * VW]
                            nc.tensor.matmul(
                                pvs[h][:], va[:],
                                pt[:, half * 512:(half + 1) * 512],
                                start=(jp == 0 and half == 0),
                                stop=(jp == NJP - 1 and half == 1),
                            )

                for jp in range(NJP):
                    emit_st(jp)
                    fill(1)
                    if jp > 1:
                        emit_pv(jp - 2)
                emit_pv(NJP - 2)
                fill(1)
                emit_pv(NJP - 1)
                if last:
                    # last q-block: skip the normalize broadcast chain — copy
                    # UNNORMALIZED O^T straight to ot and extract token-major
                    # reciprocals [128, 4] (token t of subtile s at partition
                    # t, column s); the tail's per-head output projection
                    # applies them as per-partition drain scales
                    r4bs = {}
                    for h in range(NH):
                        osl = ot_sb[h * HD:(h + 1) * HD, q0:q0 + 512]
                        nc.vector.tensor_copy(osl, pvs[h][0:HD, :])
                        m64 = npl.tile([VW, 512], dt.float32, tag="m64",
                                       bufs=4, name=f"m64_{qb}_{h}")
                        nc.vector.tensor_copy(m64[HD:VW, :], pvs[h][HD:VW, :])
                        d4b = npl.tile([128, 4], dt.float32, tag="d4b",
                                       bufs=4, name=f"d4b_{qb}_{h}")
                        engs = [nc.sync, nc.scalar, nc.gpsimd]
                        for s in range(4):
                            engs[(h * 4 + s) % 3].dma_start(
                                d4b[:, s:s + 1],
                                m64[HD:VW, s * 128:(s + 1) * 128],
                            )
                        r4b = npl.tile([128, 4], dt.float32, tag="r4b",
                                       bufs=4, name=f"r4b_{qb}_{h}")
                        nc.vector.reciprocal(r4b[:], d4b[:])
                        r4bs[h] = r4b
                    fill(100)
                    return r4bs
                # copy both heads' PV to SBUF first (frees PSUM), launch the
                # denominator-reshape DMAs immediately after each copy, and
                # only then run the reciprocals: by the time the in-order DVE
                # queue reaches recip h0, its d4 DMA has landed — the DVE
                # must never sit waiting on a DMA, since that head-of-line
                # blocking stalls the PSUM-recycling casts the PE filler
                # matmuls depend on
                ms = {}
                d4s = {}
                for h in range(NH):
                    m = npl.tile([VW, 512], dt.float32, tag="m", name=f"m_{b}_{qb}_{h}")
                    nc.vector.tensor_copy(m[:], pvs[h][:])
                    d4 = npl.tile([128, 4], dt.float32, tag="d4",
                                  name=f"d4_{b}_{qb}_{h}")
                    nc.sync.dma_start(
                        d4[:],
                        m[HD:VW, :].rearrange("p (a c) -> p a c", c=4),
                    )
                    ms[h] = m
                    d4s[h] = d4
                fill(2)
                for h in range(NH):
                    m = ms[h]
                    r4 = npl.tile([128, 4], dt.float32, tag="r4", name=f"r4_{b}_{qb}_{h}")
                    nc.vector.reciprocal(r4[:], d4s[h][:])
                    rc = npl.tile([1, 512], dt.float32, tag="rc", name=f"rc_{b}_{qb}_{h}")
                    nc.sync.dma_start(
                        rc[:].rearrange("p (a c) -> p a c", c=4), r4[:]
                    )
                    rbt = npl.tile([HD, 512], dt.float32, tag="rb", name=f"rb_{b}_{qb}_{h}")
                    nc.sync.dma_start(
                        rbt[:],
                        rc[0:1, :].rearrange("p (o q) -> p o q", o=1)
                        .broadcast_to((1, HD, 512)),
                    )
                    # bv is folded into the host-side gather (bv @ Wo^T is
                    # a constant row vector), so the normalize is a single
                    # mul. GpSimd mid-kernel (keeps the DVE queue unblocked);
                    # DVE in the last q-blocks, where the GpSimd wake-up
                    # latency would sit on the tail critical path.
                    osl = ot_sb[h * HD:(h + 1) * HD, q0:q0 + 512]
                    eng = nc.vector if late else nc.gpsimd
                    eng.tensor_mul(osl, m[0:HD, :], rbt[:])
                fill(100)   # drain any leftover fillers

            def p3_tile_scaled(g, r4bs):
                """Per-head output projection for the last q-block: the two
                64-contraction matmuls pair concurrently on the PE-array
                halves, and the softmax reciprocal is applied per-token as a
                drain scale (ACT for head 0, DVE mult-add for head 1)."""
                s = g % 4
                o_sb = op.tile([128, EMB], dt.float16, tag="osb", name=f"o_{g}")
                for ch in range(2):
                    ps0 = ps.tile([128, 512], dt.float32, tag="pp", bufs=2,
                                  name=f"opsa_{g}_{ch}")
                    ps1 = ps.tile([128, 512], dt.float32, tag="pp", bufs=2,
                                  name=f"opsb_{g}_{ch}")
                    nc.tensor.matmul(
                        ps0[:], ot_sb[0:HD, g * 128:(g + 1) * 128],
                        wo_sb[0:HD, ch * 512:(ch + 1) * 512],
                        start=True, stop=True,
                    )
                    nc.tensor.matmul(
                        ps1[:], ot_sb[HD:F, g * 128:(g + 1) * 128],
                        wo_sb[HD:F, ch * 512:(ch + 1) * 512],
                        start=True, stop=True,
                    )
                    dst = o_sb[:, ch * 512:(ch + 1) * 512]
                    nc.scalar.activation(
                        dst, ps0[:], mybir.ActivationFunctionType.Copy,
                        scale=r4bs[0][:, s:s + 1],
                    )
                    nc.vector.scalar_tensor_tensor(
                        dst, ps1[:], r4bs[1][:, s:s + 1], dst,
                        op0=mybir.AluOpType.mult, op1=mybir.AluOpType.add,
                    )
                nc.sync.dma_start(out_ext[g * 128:(g + 1) * 128, :], o_sb[:])

            # ---------------- emission schedule ----------------
            # prologue: first 3 projection blocks upfront
            for rb in range(PROLOGUE_BLOCKS):
                for f in p1_block_fillers(rb):
                    f()
            # p3 tiles per q-block: lag >= 2 q-blocks behind (so ot rows are
            # long past the normalize chain); back-load the distribution so
            # the last batch — which has no projection fillers left — still
            # keeps the PE busy.
            P3_COUNT = [0, 0, 3, 3, 3, 3, 3, 3, 3, 3, 3, 3, 6, 6, 6, 8]
            nxt_g = 0
            for b in range(B):
                for qb in range(NQB):
                    i = b * NQB + qb          # global q-block index 0..15
                    fillers = []
                    blk = PROLOGUE_BLOCKS + i  # p1 block for this q-block
                    if blk < NBLK:
                        fillers.extend(p1_block_fillers(blk))
                    hi = min(nxt_g + P3_COUNT[i], (i - 1) * 4 if i > 1 else 0)
                    for g in range(nxt_g, hi):
                        fillers.append(lambda g=g: p3_tile(g, act_drain=False))
                    nxt_g = max(nxt_g, hi)
                    r = p2_qblock(b, qb, fillers,
                                  late=(i >= NQB * B - 2),
                                  last=(i >= NQB * B - 2))
                    if r is not None:
                        r4bs_by_qb = locals().get('r4bs_by_qb') or {}
                        r4bs_by_qb[i] = r
            # tail: scaled per-head projection for the last two q-blocks
            # (their ot rows are unnormalized; the reciprocal rides the
            # drain as a per-token scale)
            for g in range(nxt_g, G):
                p3_tile_scaled(g, r4bs_by_qb[14 + (g >= G - 4)])
    return nc


_NC_CACHE = None


def _get_nc():
    global _NC_CACHE
    if _NC_CACHE is None:
        _NC_CACHE = build_bass()
    return _NC_CACHE


def make_in_maps(x, Wq, bq, Wk, bk, Wv, bv, Wo, bo):
    xt = np.ascontiguousarray(
        np.asarray(x, dtype=np.float32).reshape(R, EMB).astype(bf16).T
    )
    in_maps = []
    for c in range(NCORES):
        rows = slice(F * c, F * (c + 1))
        in_maps.append({
            "xt": xt,
            "wq": np.ascontiguousarray(np.asarray(Wq)[rows, :].T.astype(bf16)),
            "wk": np.ascontiguousarray(np.asarray(Wk)[rows, :].T.astype(bf16)),
            "wv": np.ascontiguousarray(np.asarray(Wv)[rows, :].T.astype(bf16)),
            "wo": np.ascontiguousarray(np.asarray(Wo)[:, rows].T.astype(bf16)),
            "bq": np.asarray(bq)[rows].reshape(F, 1).astype(np.float32),
        })
    return in_maps


def gather(results, bv, Wo, bo):
    acc = np.zeros((R, EMB), np.float32)
    for r in results:
        acc += r["out"].astype(np.float32)
    # bv rides through the output projection as a constant row vector
    acc += np.asarray(bo, dtype=np.float32) + (
        np.asarray(bv, dtype=np.float64) @ np.asarray(Wo, dtype=np.float64).T
    ).astype(np.float32)
    return acc.reshape(B, T, EMB)


def kernel(x, Wq, bq, Wk, bk, Wv, bv, Wo, bo, _trace=False):
    nc = _get_nc()
    in_maps = make_in_maps(x, Wq, bq, Wk, bk, Wv, bv, Wo, bo)
    res = run_bass_kernel_spmd(nc, in_maps, list(range(NCORES)), trace=_trace)
    out = gather(res.results, bv, Wo, bo)
    if _trace:
        kernel.last_result = res
    return out
